# revision 17
# baseline (speedup 1.0000x reference)
"""ALSH-AlexNet on 8 TRN2 NeuronCores.

Strategy:
- Host: gather weights by the runtime index sets (idx1..idx5), phase-expand the
  conv1 input, shard batch 64 -> 8 images/core, shard fc6/fc7/fc8 over the
  output dim (tensor-parallel).
- Device (SPMD, identical program): conv stack data-parallel per core in
  float32r (TF32-like matmul at 1 cycle/row), fused maxpools on DVE, then
  AllGather of the pooled features and tensor-parallel FC layers in bf16.
- Host: concatenate the 8 cores' [125, 64] fc8 shards -> [64, 1000].

The NEFF is input-independent (indices are applied on host), so build+compile
is cached at module level.
"""
import os
import numpy as np
import ml_dtypes

import concourse.bass as bass
import concourse.bacc as bacc
import concourse.mybir as mybir
import concourse.tile as tile
from concourse.bass_utils import run_bass_kernel_spmd

F32R = mybir.dt.float32r
F32 = mybir.dt.float32
BF16 = mybir.dt.bfloat16
AF = mybir.ActivationFunctionType
AX = mybir.AxisListType
ALU = mybir.AluOpType

NCORES = 8
IMGS = 8          # images per core
BF = ml_dtypes.bfloat16

# conv1 tap order: t enumerates (qx, px) with dx = 4*qx + px <= 10
C1_TAPS = [(qx, px) for qx in range(3) for px in range(4) if 4 * qx + px <= 10]


def _install_ntff_hook():
    """Make run_bass_kernel_spmd(trace=True) work under axon."""
    import sys, types
    if "antenv.axon_hooks" in sys.modules:
        return
    mod = types.ModuleType("antenv.axon_hooks")
    mod._hook = None
    mod.set_axon_ntff_profile_hook = lambda h: setattr(mod, "_hook", h)
    mod.get_axon_ntff_profile_hook = lambda: mod._hook
    sys.modules["antenv.axon_hooks"] = mod
    import antenv
    antenv.axon_hooks = mod
    try:
        from trn_agent_boot.trn_boot import _ntff_profile_via_ctypes
        mod.set_axon_ntff_profile_hook(
            _ntff_profile_via_ctypes("/opt/axon/libaxon_pjrt.so"))
    except Exception:
        pass


def build():
    nc = bacc.Bacc(None, target_bir_lowering=False)

    xp = nc.dram_tensor("xp", [IMGS, 33, 4, 55, 57], F32R, kind="ExternalInput")
    w1 = nc.dram_tensor("w1", [33, 11, 64], F32R, kind="ExternalInput")
    w2 = nc.dram_tensor("w2", [64, 25, 170], F32R, kind="ExternalInput")
    w3 = nc.dram_tensor("w3", [170, 9, 256], F32R, kind="ExternalInput")
    w4 = nc.dram_tensor("w4", [256, 9, 256], F32R, kind="ExternalInput")
    w5 = nc.dram_tensor("w5", [256, 9, 170], F32R, kind="ExternalInput")
    fc6w = nc.dram_tensor("fc6w", [48, 128, 512], BF16, kind="ExternalInput")
    fc7w = nc.dram_tensor("fc7w", [32, 128, 512], BF16, kind="ExternalInput")
    fc8w = nc.dram_tensor("fc8w", [32, 128, 125], BF16, kind="ExternalInput")
    b1 = nc.dram_tensor("b1", [64], F32, kind="ExternalInput")
    b2 = nc.dram_tensor("b2", [170], F32, kind="ExternalInput")
    b3 = nc.dram_tensor("b3", [256], F32, kind="ExternalInput")
    b4 = nc.dram_tensor("b4", [256], F32, kind="ExternalInput")
    b5 = nc.dram_tensor("b5", [170], F32, kind="ExternalInput")
    fc6b = nc.dram_tensor("fc6b", [512], F32, kind="ExternalInput")
    fc7b = nc.dram_tensor("fc7b", [512], F32, kind="ExternalInput")
    fc8b = nc.dram_tensor("fc8b", [125], F32, kind="ExternalInput")
    ident = nc.dram_tensor("ident", [64, 64], BF16, kind="ExternalInput")
    out = nc.dram_tensor("out", [125, 64], F32, kind="ExternalOutput")

    with tile.TileContext(nc) as tc:
        with (
            tc.tile_pool(name="wp", bufs=1) as wp,        # persistent weights
            tc.tile_pool(name="act", bufs=1) as act,      # persistent activations
            tc.tile_pool(name="bandp", bufs=2) as bandp,  # conv1 input bands
            tc.tile_pool(name="fcs", bufs=2) as fcs,      # streamed fc weights
            tc.tile_pool(name="dram", bufs=1, space="DRAM") as dram,
        ):
            # ---- load weights/biases (resident) ----
            w1_sb = wp.tile([128, 11, 64], F32R)
            nc.sync.dma_start(w1_sb[0:33], w1[:])
            nc.sync.dma_start(w1_sb[64:97], w1[:])
            w2_sb = wp.tile([128, 25, 170], F32R)
            nc.sync.dma_start(w2_sb[0:64], w2[:])
            nc.sync.dma_start(w2_sb[64:128], w2[:])
            w3a_sb = wp.tile([128, 9, 256], F32R)
            w3b_sb = wp.tile([42, 9, 256], F32R)
            nc.sync.dma_start(w3a_sb[:], w3[0:128])
            nc.sync.dma_start(w3b_sb[:], w3[128:170])
            w4a_sb = wp.tile([128, 9, 256], F32R)
            w4b_sb = wp.tile([128, 9, 256], F32R)
            nc.sync.dma_start(w4a_sb[:], w4[0:128])
            nc.sync.dma_start(w4b_sb[:], w4[128:256])
            w5a_sb = wp.tile([128, 9, 170], F32R)
            w5b_sb = wp.tile([128, 9, 170], F32R)
            nc.sync.dma_start(w5a_sb[:], w5[0:128])
            nc.sync.dma_start(w5b_sb[:], w5[128:256])

            b1_sb = wp.tile([128, 1], F32)
            nc.sync.dma_start(b1_sb[0:64], b1.ap().unsqueeze(1))
            nc.sync.dma_start(b1_sb[64:128], b1.ap().unsqueeze(1))
            b2a_sb = wp.tile([128, 1], F32)
            b2b_sb = wp.tile([42, 1], F32)
            nc.sync.dma_start(b2a_sb[:], b2.ap()[0:128].unsqueeze(1))
            nc.sync.dma_start(b2b_sb[:], b2.ap()[128:170].unsqueeze(1))
            b3_sb = wp.tile([128, 2], F32)
            nc.sync.dma_start(b3_sb[:], b3.ap().rearrange("(a p) -> p a", a=2))
            b4_sb = wp.tile([128, 2], F32)
            nc.sync.dma_start(b4_sb[:], b4.ap().rearrange("(a p) -> p a", a=2))
            b5a_sb = wp.tile([128, 1], F32)
            b5b_sb = wp.tile([42, 1], F32)
            nc.sync.dma_start(b5a_sb[:], b5.ap()[0:128].unsqueeze(1))
            nc.sync.dma_start(b5b_sb[:], b5.ap()[128:170].unsqueeze(1))
            fc6b_sb = wp.tile([128, 4], F32)
            nc.sync.dma_start(fc6b_sb[:], fc6b.ap().rearrange("(a p) -> p a", a=4))
            fc7b_sb = wp.tile([128, 4], F32)
            nc.sync.dma_start(fc7b_sb[:], fc7b.ap().rearrange("(a p) -> p a", a=4))
            fc8b_sb = wp.tile([125, 1], F32)
            nc.sync.dma_start(fc8b_sb[:], fc8b.ap().unsqueeze(1))
            ident_sb = wp.tile([64, 64], BF16)
            nc.sync.dma_start(ident_sb[:], ident[:])

            # ---- persistent activation buffers (ping-pong where needed) ----
            # pool1t: padded 31x31 (pad=2) per image half (A: part 0-63, B: 64-127)
            pool1ts = [act.tile([128, 968], F32R, name=f"pool1t{i}", tag=f"pool1t{i}") for i in range(2)]
            # pool2t/c3t/c4t: 2-image frames of 225 + 32 slack
            p2a = [act.tile([128, 482], F32R, name=f"p2a{i}", tag=f"p2a{i}") for i in range(2)]
            p2b = [act.tile([42, 482], F32R, name=f"p2b{i}", tag=f"p2b{i}") for i in range(2)]
            c3a = [act.tile([128, 482], F32R, name=f"c3a{i}", tag=f"c3a{i}") for i in range(2)]
            c3b = [act.tile([128, 482], F32R, name=f"c3b{i}", tag=f"c3b{i}") for i in range(2)]
            c4a = [act.tile([128, 482], F32R, name=f"c4a{i}", tag=f"c4a{i}") for i in range(2)]
            c4b = [act.tile([128, 482], F32R, name=f"c4b{i}", tag=f"c4b{i}") for i in range(2)]
            # zero-init the padded f32r buffers (memset can't write f32r;
            # DVE copy from an f32 zeros tile performs the f32r rounding)
            zf = act.tile([128, 968], F32)
            nc.vector.memset(zf[:], 0.0)
            for t in pool1ts + p2a + p2b + c3a + c3b + c4a + c4b:
                tp = t[:]
                nc.vector.tensor_copy(tp, zf[0:tp.shape[0], 0:tp.shape[1]])

            f_bfa = act.tile([128, 36, IMGS], BF16)   # features ch 0-127, [c,s,img]
            f_bfb = act.tile([42, 36, IMGS], BF16)    # features ch 128-169
            ag1_in = dram.tile([6120, 8], BF16)

            with tc.tile_pool(name="cps", bufs=4, space="PSUM") as cps, \
                 tc.tile_pool(name="scratch", bufs=2) as scr:
                for pair in range(IMGS // 2):
                    pp = pair % 2
                    pool1t, p2ta, p2tb = pool1ts[pp], p2a[pp], p2b[pp]
                    c3ta, c3tb, c4ta, c4tb = c3a[pp], c3b[pp], c4a[pp], c4b[pp]
                    imA, imB = 2 * pair, 2 * pair + 1

                    # ======== conv1 (stride 4) + pool1, imgs A/B packed in rows
                    # A uses PE rows 0:33, B rows 64:97 (concurrent); both
                    # write their own PSUM bank at partitions 0:64 (matmul PSUM
                    # dst must start at partition 0). B's pooled result is
                    # DMA-shifted to partitions 64:128 of pool1t afterwards.
                    htmpA = scr.tile([64, 55, 27], F32, tag="htmpA")
                    htmpB = scr.tile([64, 55, 27], F32, tag="htmpB")
                    for r in range(7):
                        y0, ny = 8 * r, min(8, 55 - 8 * r)
                        ne = ny * 57 + (ny * 57) % 2
                        band = bandp.tile([128, 1832], F32R, tag="band")
                        for im, p0 in ((imA, 0), (imB, 64)):
                            bdst = bass.AP(band.tensor, p0 * 1832,
                                           [[1832, 33], [456, 4], [57, ny], [1, 57]])
                            nc.sync.dma_start(bdst, xp[im, :, :, y0:y0 + ny, :])
                        psA = cps.tile([64, 456], F32, tag="pa")
                        psB = cps.tile([64, 456], F32, tag="pb")
                        for t, (qx, px) in enumerate(C1_TAPS):
                            off = px * 456 + qx
                            nc.tensor.matmul(
                                psA[:, :ne], w1_sb[0:33, t, :],
                                band[0:33, off:off + ne],
                                start=(t == 0), stop=(t == 10))
                            nc.tensor.matmul(
                                psB[:, :ne], w1_sb[64:97, t, :],
                                band[64:97, off:off + ne],
                                start=(t == 0), stop=(t == 10))
                        for ps_t, ht in ((psA, htmpA), (psB, htmpB)):
                            hsrc = bass.AP(ps_t.tensor, 0,
                                           [[456, 64], [57, ny], [2, 27], [1, 3]])
                            nc.vector.tensor_reduce(
                                ht[:, y0:y0 + ny, :], hsrc,
                                axis=AX.X, op=ALU.max)

                    # pool1 v-pass + bias; A written in place, B via DMA shift
                    vtmpA = scr.tile([64, 27, 27], F32, tag="vtmpA")
                    vsrcA = bass.AP(htmpA.tensor, 0,
                                    [[55 * 27, 64], [54, 27], [1, 27], [27, 3]])
                    nc.vector.tensor_reduce(vtmpA[:], vsrcA, axis=AX.X, op=ALU.max)
                    p1dstA = bass.AP(pool1t.tensor, 2 * 31 + 2,
                                     [[968, 64], [31, 27], [1, 27]])
                    nc.scalar.activation(p1dstA, vtmpA[:], AF.Identity,
                                         bias=b1_sb[0:64, 0:1])
                    vtmpB = scr.tile([64, 27, 27], F32, tag="vtmpB")
                    vsrcB = bass.AP(htmpB.tensor, 0,
                                    [[55 * 27, 64], [54, 27], [1, 27], [27, 3]])
                    nc.vector.tensor_reduce(vtmpB[:], vsrcB, axis=AX.X, op=ALU.max)
                    vtmpBr = scr.tile([64, 729], F32R, tag="vtmpBr")
                    nc.scalar.activation(vtmpBr[:],
                                         vtmpB[:].rearrange("p a b -> p (a b)"),
                                         AF.Identity, bias=b1_sb[0:64, 0:1])
                    p1dstB = bass.AP(pool1t.tensor, 64 * 968 + 2 * 31 + 2,
                                     [[968, 64], [31, 27], [1, 27]])
                    nc.sync.dma_start(p1dstB, vtmpBr[:])

                    # ======== conv2 (pad 2) + pool2, A/B packed in rows
                    htmp2 = [scr.tile([128, 27, 13], F32, tag=f"h2_{i}", name=f"h2_{i}")
                             for i in range(2)]          # M1 chunks, per image
                    htmp2b = [scr.tile([42, 27, 13], F32, tag=f"h2b_{i}", name=f"h2b_{i}")
                              for i in range(2)]         # M2 chunks, per image
                    for half, p0 in ((0, 0), (1, 64)):
                        for mi, (m0, mw) in enumerate(((0, 128), (128, 42))):
                            for y0, nyr in ((0, 16), (16, 11)):
                                ne = nyr * 31 + (nyr * 31) % 2
                                psum = cps.tile([128, 496], F32,
                                                tag="pa" if half == 0 else "pb")
                                for t in range(25):
                                    dy, dx = divmod(t, 5)
                                    off = (y0 + dy) * 31 + dx
                                    rhs = pool1t[p0:p0 + 64, off:off + ne]
                                    nc.tensor.matmul(
                                        psum[:mw, :ne],
                                        w2_sb[p0:p0 + 64, t, m0:m0 + mw], rhs,
                                        start=(t == 0), stop=(t == 24))
                                dst = (htmp2 if mi == 0 else htmp2b)[half]
                                hsrc = bass.AP(psum.tensor, 0,
                                               [[496, mw], [31, nyr], [2, 13], [1, 3]])
                                nc.vector.tensor_reduce(
                                    dst[:mw, y0:y0 + nyr, :], hsrc,
                                    axis=AX.X, op=ALU.max)
                    # pool2 v-pass + bias -> p2t frames (pad=1 -> 15x15)
                    for half in range(2):
                        for src_t, dst_t, mw, bias in (
                                (htmp2[half], p2ta, 128, b2a_sb),
                                (htmp2b[half], p2tb, 42, b2b_sb)):
                            vsrc = bass.AP(src_t.tensor, 0,
                                           [[27 * 13, mw], [26, 13], [1, 13], [13, 3]])
                            vt = scr.tile([128, 13, 13], F32, tag="vt2")
                            nc.vector.tensor_reduce(vt[:mw], vsrc,
                                                    axis=AX.X, op=ALU.max)
                            dst = bass.AP(dst_t.tensor, half * 225 + 16,
                                          [[482, mw], [15, 13], [1, 13]])
                            nc.scalar.activation(dst, vt[:mw], AF.Identity,
                                                 bias=bias[:, 0:1])

                    # ======== conv3 (pad 1): 170 -> 256, 2-img frames N=450
                    for mi, m0 in ((0, 0), (1, 128)):
                        psum = cps.tile([128, 456], F32, tag="pa" if mi == 0 else "pb")
                        t = 0
                        for dy in range(3):
                            for dx in range(3):
                                off = dy * 15 + dx
                                nc.tensor.matmul(
                                    psum[:, :450], w3a_sb[:, 3 * dy + dx, m0:m0 + 128],
                                    p2ta[:, off:off + 450],
                                    start=(t == 0), stop=False)
                                t += 1
                                nc.tensor.matmul(
                                    psum[:, :450], w3b_sb[:, 3 * dy + dx, m0:m0 + 128],
                                    p2tb[0:42, off:off + 450],
                                    start=False, stop=(t == 17))
                                t += 1
                        # evac valid 13x13 of each frame + bias -> c3t interior
                        dst_t = c3ta if mi == 0 else c3tb
                        src = bass.AP(psum.tensor, 0,
                                      [[456, 128], [225, 2], [15, 13], [1, 13]])
                        dst = bass.AP(dst_t.tensor, 16,
                                      [[482, 128], [225, 2], [15, 13], [1, 13]])
                        nc.scalar.activation(dst, src, AF.Identity,
                                             bias=b3_sb[:, mi:mi + 1])

                    # ======== conv4: 256 -> 256
                    for mi, m0 in ((0, 0), (1, 128)):
                        psum = cps.tile([128, 456], F32, tag="pa" if mi == 0 else "pb")
                        t = 0
                        for dy in range(3):
                            for dx in range(3):
                                off = dy * 15 + dx
                                nc.tensor.matmul(
                                    psum[:, :450], w4a_sb[:, 3 * dy + dx, m0:m0 + 128],
                                    c3ta[:, off:off + 450],
                                    start=(t == 0), stop=False)
                                t += 1
                                nc.tensor.matmul(
                                    psum[:, :450], w4b_sb[:, 3 * dy + dx, m0:m0 + 128],
                                    c3tb[:, off:off + 450],
                                    start=False, stop=(t == 17))
                                t += 1
                        dst_t = c4ta if mi == 0 else c4tb
                        src = bass.AP(psum.tensor, 0,
                                      [[456, 128], [225, 2], [15, 13], [1, 13]])
                        dst = bass.AP(dst_t.tensor, 16,
                                      [[482, 128], [225, 2], [15, 13], [1, 13]])
                        nc.scalar.activation(dst, src, AF.Identity,
                                             bias=b4_sb[:, mi:mi + 1])

                    # ======== conv5: 256 -> 170, + pool3 + bias -> features
                    for mi, (m0, mw, bias, fdst) in enumerate((
                            (0, 128, b5a_sb, f_bfa), (128, 42, b5b_sb, f_bfb))):
                        psum = cps.tile([128, 456], F32, tag="pa" if mi == 0 else "pb")
                        t = 0
                        for dy in range(3):
                            for dx in range(3):
                                off = dy * 15 + dx
                                nc.tensor.matmul(
                                    psum[:mw, :450], w5a_sb[:, 3 * dy + dx, m0:m0 + mw],
                                    c4ta[:, off:off + 450],
                                    start=(t == 0), stop=False)
                                t += 1
                                nc.tensor.matmul(
                                    psum[:mw, :450], w5b_sb[:, 3 * dy + dx, m0:m0 + mw],
                                    c4tb[:, off:off + 450],
                                    start=False, stop=(t == 17))
                                t += 1
                        # pool3 h+v per image, then bias -> bf16 features
                        h3 = scr.tile([128, 2, 13, 6], F32, tag="h3")
                        v3 = scr.tile([128, 2, 6, 6], F32, tag="v3")
                        for im in range(2):
                            hsrc = bass.AP(psum.tensor, im * 225,
                                           [[456, mw], [15, 13], [2, 6], [1, 3]])
                            nc.vector.tensor_reduce(h3[:mw, im], hsrc,
                                                    axis=AX.X, op=ALU.max)
                            vsrc = bass.AP(h3.tensor, im * 78,
                                           [[2 * 78, mw], [12, 6], [1, 6], [6, 3]])
                            nc.vector.tensor_reduce(v3[:mw, im], vsrc,
                                                    axis=AX.X, op=ALU.max)
                        fdap = bass.AP(fdst.tensor, imA,
                                       [[36 * IMGS, mw], [IMGS, 36], [1, 2]])
                        vsrc2 = bass.AP(v3.tensor, 0,
                                        [[72, mw], [1, 36], [36, 2]])
                        nc.scalar.activation(fdap, vsrc2, AF.Identity,
                                             bias=bias[:, 0:1])
                        # stream this pair's feature columns into the AG payload
                        base = 0 if mi == 0 else 128 * 36 * 8
                        pd = bass.AP(ag1_in.tensor, base + imA,
                                     [[36 * 8, mw], [8, 36], [1, 2]])
                        ps_src = bass.AP(fdst.tensor, imA,
                                         [[36 * IMGS, mw], [IMGS, 36], [1, 2]])
                        nc.sync.dma_start(pd, ps_src)

            # ======== feature AllGather (payload written during conv)
            ag1_out = dram.tile([NCORES * 6120, 8], BF16, addr_space="Shared")
            nc.gpsimd.collective_compute(
                "AllGather", ALU.bypass,
                replica_groups=[list(range(NCORES))],
                ins=[ag1_in[:].opt()], outs=[ag1_out[:].opt()])

            # fT chunks: 4 tiles of 12 K-chunks so fc6 can start early
            fTs = [act.tile([128, 12, 64], BF16, name=f"fT{i}", tag=f"fT{i}")
                   for i in range(4)]
            for q in range(48):
                rows = min(128, 6120 - q * 128)
                src = bass.AP(ag1_out.tensor, q * 128 * 8,
                              [[8, rows], [6120 * 8, NCORES], [1, IMGS]])
                eng = nc.sync if q % 2 == 0 else nc.gpsimd
                eng.dma_start(fTs[q // 12][:rows, q % 12, :], src)

            with tc.tile_pool(name="fps", bufs=1, space="PSUM") as fps, \
                 tc.tile_pool(name="ftp", bufs=2, space="PSUM") as ftp:
                # fc6/fc7 run with the activations stationary and the weight
                # matrix moving (N=512) -> one PSUM bank, no LDW bottleneck.
                # Output [64 img, 512 feat] is PE-transposed back to
                # [feat, img] for the next layer's AllGather.
                ps6 = fps.tile([64, 512], F32, name="ps6")
                for q in range(48):
                    rows = min(128, 6120 - q * 128)
                    nc.tensor.matmul(
                        ps6[:, :], fTs[q // 12][:rows, q % 12, :],
                        fc6w_sb[:rows, q, :],
                        start=(q == 0), stop=(q == 47))
                fc6r = act.tile([64, 512], BF16)
                nc.scalar.activation(fc6r[:], ps6[:], AF.Copy)
                fc6o = act.tile([128, 4, 64], BF16)
                for m in range(4):
                    pst = ftp.tile([128, 64], BF16, tag="pst", name="pst")
                    nc.tensor.transpose(pst[:], fc6r[:, 128 * m:128 * m + 128],
                                        ident_sb[:])
                    nc.scalar.activation(fc6o[:, m, :], pst[:], AF.Identity,
                                         bias=fc6b_sb[:, m:m + 1])

                ag2_in = dram.tile([512, 64], BF16)
                ag2_out = dram.tile([NCORES * 512, 64], BF16, addr_space="Shared")
                d = bass.AP(ag2_in.tensor, 0, [[64, 128], [128 * 64, 4], [1, 64]])
                nc.sync.dma_start(d, fc6o[:])
                nc.gpsimd.collective_compute(
                    "AllGather", ALU.bypass,
                    replica_groups=[list(range(NCORES))],
                    ins=[ag2_in[:].opt()], outs=[ag2_out[:].opt()])
                fc7in = act.tile([128, 32, 64], BF16)
                sIn = bass.AP(ag2_out.tensor, 0, [[64, 128], [128 * 64, 32], [1, 64]])
                nc.sync.dma_start(fc7in[:], sIn)

                ps7 = fps.tile([64, 512], F32, name="ps7")
                for q in range(32):
                    nc.tensor.matmul(
                        ps7[:, :], fc7in[:, q, :], fc7w_sb[:, q, :],
                        start=(q == 0), stop=(q == 31))
                fc7r = act.tile([64, 512], BF16)
                nc.scalar.activation(fc7r[:], ps7[:], AF.Copy)
                fc7o = act.tile([128, 4, 64], BF16)
                for m in range(4):
                    pst = ftp.tile([128, 64], BF16, tag="pst", name="pst")
                    nc.tensor.transpose(pst[:], fc7r[:, 128 * m:128 * m + 128],
                                        ident_sb[:])
                    nc.scalar.activation(fc7o[:, m, :], pst[:], AF.Identity,
                                         bias=fc7b_sb[:, m:m + 1])

                ag3_in = dram.tile([512, 64], BF16)
                ag3_out = dram.tile([NCORES * 512, 64], BF16, addr_space="Shared")
                d = bass.AP(ag3_in.tensor, 0, [[64, 128], [128 * 64, 4], [1, 64]])
                nc.sync.dma_start(d, fc7o[:])
                nc.gpsimd.collective_compute(
                    "AllGather", ALU.bypass,
                    replica_groups=[list(range(NCORES))],
                    ins=[ag3_in[:].opt()], outs=[ag3_out[:].opt()])
                fc8in = act.tile([128, 32, 64], BF16)
                sIn = bass.AP(ag3_out.tensor, 0, [[64, 128], [128 * 64, 32], [1, 64]])
                nc.sync.dma_start(fc8in[:], sIn)

                # fc8 keeps weights stationary (psum [125, 64], per-partition bias)
                ps8 = fps.tile([128, 64], F32, name="ps8")
                for q in range(32):
                    nc.tensor.matmul(
                        ps8[:125, :], fc8w_sb[:, q, :], fc8in[:, q, :],
                        start=(q == 0), stop=(q == 31))
                out_sb = act.tile([125, 64], F32)
                nc.scalar.activation(out_sb[:], ps8[:125, :], AF.Identity,
                                     bias=fc8b_sb[:, 0:1])
                nc.sync.dma_start(out[:], out_sb[:])

    nc.finalize()
    return nc


_NC_CACHE = {}


def _get_nc():
    if "nc" not in _NC_CACHE:
        _NC_CACHE["nc"] = build()
    return _NC_CACHE["nc"]


def _expand_conv1(x):
    """x [N,3,227,227] f32 -> [N, 33, 4, 55, 57]: [(c,dy), px, y, x']."""
    n = x.shape[0]
    xp = np.zeros((n, 3, 11, 4, 55, 57), np.float32)
    for dy in range(11):
        rows = x[:, :, dy::4, :][:, :, :55, :]          # [n,3,55,227]
        for px in range(4):
            cols = rows[:, :, :, px::4]                 # [n,3,55,57 or 56]
            xp[:, :, dy, px, :, :cols.shape[3]] = cols
    return xp.reshape(n, 33, 4, 55, 57)


def kernel(x, idx1, idx2, idx3, idx4, idx5,
           W1, b1, W2, b2, W3, b3, W4, b4, W5, b5,
           fc6_w, fc6_b, fc7_w, fc7_b, fc8_w, fc8_b):
    x = np.asarray(x, np.float32)
    idx1 = np.asarray(idx1).astype(np.int64)
    idx2 = np.asarray(idx2).astype(np.int64)
    idx3 = np.asarray(idx3).astype(np.int64)
    idx4 = np.asarray(idx4).astype(np.int64)
    idx5 = np.asarray(idx5).astype(np.int64)

    # ---- host routing: gather active filters / input channels ----
    W1a = np.asarray(W1, np.float32)[idx1]                       # [64,3,11,11]
    W2a = np.asarray(W2, np.float32)[idx2][:, idx1]              # [170,64,5,5]
    W3a = np.asarray(W3, np.float32)[idx3][:, idx2]              # [256,170,3,3]
    W4a = np.asarray(W4, np.float32)[idx4][:, idx3]              # [256,256,3,3]
    W5a = np.asarray(W5, np.float32)[idx5][:, idx4]              # [170,256,3,3]
    b1a = np.asarray(b1, np.float32)[idx1]
    b2a = np.asarray(b2, np.float32)[idx2]
    b3a = np.asarray(b3, np.float32)[idx3]
    b4a = np.asarray(b4, np.float32)[idx4]
    b5a = np.asarray(b5, np.float32)[idx5]
    # fc6 rows for active ch of pool3 output (zero-fill scatter == row gather)
    fc6_wa = np.asarray(fc6_w, np.float32).reshape(256, 36, 4096)[idx5]
    fc6_wa = fc6_wa.reshape(6120, 4096)

    # ---- device weight layouts ----
    w1dev = np.zeros((3, 11, 11, 64), np.float32)
    for t, (qx, px) in enumerate(C1_TAPS):
        w1dev[:, :, t, :] = np.transpose(W1a[:, :, :, 4 * qx + px], (1, 2, 0))
    w1dev = w1dev.reshape(33, 11, 64)
    w2dev = np.ascontiguousarray(
        np.transpose(W2a, (1, 2, 3, 0)).reshape(64, 25, 170))
    w3dev = np.ascontiguousarray(
        np.transpose(W3a, (1, 2, 3, 0)).reshape(170, 9, 256))
    w4dev = np.ascontiguousarray(
        np.transpose(W4a, (1, 2, 3, 0)).reshape(256, 9, 256))
    w5dev = np.ascontiguousarray(
        np.transpose(W5a, (1, 2, 3, 0)).reshape(256, 9, 170))

    fc6_pad = np.zeros((6144, 4096), np.float32)
    fc6_pad[:6120] = fc6_wa
    fc7_f = np.asarray(fc7_w, np.float32)
    fc8_f = np.asarray(fc8_w, np.float32)
    fc6b_f = np.asarray(fc6_b, np.float32)
    fc7b_f = np.asarray(fc7_b, np.float32)
    fc8b_f = np.asarray(fc8_b, np.float32)

    xp = _expand_conv1(x).reshape(NCORES, IMGS, 33, 4, 55, 57)

    in_maps = []
    for c in range(NCORES):
        mo, m8 = 512 * c, 125 * c
        in_maps.append({
            "xp": xp[c],
            "w1": w1dev, "w2": w2dev, "w3": w3dev, "w4": w4dev, "w5": w5dev,
            "b1": b1a, "b2": b2a, "b3": b3a, "b4": b4a, "b5": b5a,
            "fc6w": np.ascontiguousarray(
                fc6_pad[:, mo:mo + 512]).astype(BF).reshape(48, 128, 512),
            "fc7w": np.ascontiguousarray(
                fc7_f[:, mo:mo + 512]).astype(BF).reshape(32, 128, 512),
            "fc8w": np.ascontiguousarray(
                fc8_f[:, m8:m8 + 125]).astype(BF).reshape(32, 128, 125),
            "fc6b": np.ascontiguousarray(fc6b_f[mo:mo + 512]),
            "fc7b": np.ascontiguousarray(fc7b_f[mo:mo + 512]),
            "fc8b": np.ascontiguousarray(fc8b_f[m8:m8 + 125]),
            "ident": np.eye(64, dtype=BF),
        })

    nc = _get_nc()
    trace = bool(os.environ.get("ALSH_TRACE"))
    if trace:
        _install_ntff_hook()
    r = run_bass_kernel_spmd(nc, in_maps, core_ids=list(range(NCORES)),
                             trace=trace)
    if trace and r.exec_time_ns is not None:
        print(f"HW exec time: {r.exec_time_ns} ns")
        if r.instructions_and_trace:
            print("trace:", r.instructions_and_trace[1])

    # assemble [64, 1000]
    blocks = [r.results[c]["out"] for c in range(NCORES)]   # each [125, 64]
    return np.ascontiguousarray(np.concatenate(blocks, axis=0).T)
